# revision 26
# baseline (speedup 1.0000x reference)
"""GCNConv (dense normalized adjacency) on 8 Trainium2 NeuronCores.

out = D^-1/2 (A + I, deduped) D^-1/2 @ x @ W

Strategy (1D row partition of N across 8 cores, per the sharding hint:
"shard rows of the adjacency and x across devices, replicate the
256x256 weight"):
  - Host: dedup edges + self-loops and encode the binary adjacency
    slice for each core as a dense bf16 0/1 matrix (A^T column-slice,
    [N, 1280]) — an index-data re-encoding (0/1 bitmap) of edge_index,
    no float math. Degrees are integer edge counts (bincount).
    The on-device scatter path (indirect DMA / dma_gather) is precluded
    by this image's neuronxcc: InstDMAGatherAnt fails codegen ("ISA
    wrong length") and per-row indirect DMA issues at ~1.6us/instr.
  - Device (per core, rows r in [1250*m, 1250*m+1250)):
      * dinv = sqrt(1/deg) on DVE+ACT
      * xs = dinv * x, cast to bf16, resident in SBUF
      * stream A^T slice chunks [128, 1280] bf16 from HBM (sequential,
        full DMA bandwidth), aggregate Y[rowtile] += A^T_chunk.T @ xs
        on the PE (79 chunks x 10 rowtiles, fp32 PSUM accumulation)
      * outer scale by dinv[row], PE transpose, project with the
        replicated 256x256 W in fp32, DMA out each 128-row tile.
"""

import os
import sys

sys.path.insert(0, "/opt/trn_rl_repo")
os.environ.setdefault("MYCRO_LOCAL_CACHE", "1")

import numpy as np

N = 10000
NPAD = 10112        # 79 * 128
KC = 79             # node chunks of 128
CIN = 256
COUT = 256
NCORES = 8
RPC = 1250          # valid rows per core
RTILE = 1280        # padded rows per core (10 tiles of 128)
RT = 10
P = 128

_BUILD_CACHE = {}


def _make_tc(nc):
    """TileContext whose instructions carry at most one sync-wait each.

    This image's walrus build encodes a single sync-wait command per
    instruction ("Too many sync wait commands" otherwise); Tile's
    semaphore pass freely attaches several. Split the extras onto
    freshly inserted same-engine nops placed just before the carrying
    instruction, and do the same for the kernel-tail drain.
    """
    import concourse.tile as tile
    from concourse import mybir
    from concourse.vector_clock import ScopedClock

    MAXW = 1

    def make_carrier(engine, chunk):
        nop = mybir.InstNoOp(name=nc.get_next_instruction_name(),
                             ins=[], outs=[])
        nop.engine = engine
        nop.sync_info = mybir.SyncInfo(on_wait=list(chunk), on_update=[])
        return nop

    class TC(tile.TileContext):
        def _add_instruction(self, inst):
            si = getattr(inst, "sync_info", None)
            eng = getattr(inst, "engine", mybir.EngineType.Unassigned)
            if (si is not None and si.on_wait and len(si.on_wait) > MAXW
                    and eng != mybir.EngineType.Unassigned):
                waits = list(si.on_wait)
                movable = [w for w in waits
                           if getattr(w, "wait_reg", None) is None]
                pinned = [w for w in waits
                          if getattr(w, "wait_reg", None) is not None]
                ordered = pinned + movable
                keep, extra = ordered[:MAXW], ordered[MAXW:]
                assert all(getattr(w, "wait_reg", None) is None
                           for w in extra)
                si.on_wait = keep
                for j in range(0, len(extra), MAXW):
                    super()._add_instruction(
                        make_carrier(eng, extra[j:j + MAXW]))
            super()._add_instruction(inst)

        def _drain_and_barrier(self, tick_clock, wait_clock):
            drain_inst = nc.sync.drain()
            wait_clock.add_sem_waits(
                drain_inst.ins, ScopedClock({None: tick_clock.global_clock}))
            raw = drain_inst.ins
            si = raw.sync_info
            waits = list(si.on_wait) if si and si.on_wait else []
            if len(waits) > MAXW:
                bb = nc.cur_bb.bb
                insts = list(bb.instructions)
                assert insts and insts[-1].name == raw.name
                keep, extra = waits[:MAXW], waits[MAXW:]
                si.on_wait = keep
                carriers = []
                for j in range(0, len(extra), MAXW):
                    nop = make_carrier(raw.engine, extra[j:j + MAXW])
                    nc.register_instruction(nop, overwrite=True)
                    carriers.append(nop)
                bb.instructions = insts[:-1] + carriers + [raw]
            nc.all_engine_barrier()
            assert self.sems is not None
            popped = nc._tile_sem_poison_stack.pop()
            assert popped is self._sem_poison
            nc.clear_and_free_semaphores(
                list(self.sems.allocated().values()))
            nc.all_engine_barrier()

    return TC(nc)


def _build():
    import concourse.bass as bass
    from concourse import mybir

    f32 = mybir.dt.float32
    f32r = mybir.dt.float32r
    bf16 = mybir.dt.bfloat16
    fp8 = mybir.dt.float8e4

    nc = bass.Bass("TRN2")

    at = nc.dram_tensor("at", [NPAD, RTILE], fp8, kind="ExternalInput")
    xbf = nc.dram_tensor("xbf", [NPAD, CIN], bf16, kind="ExternalInput")
    # host-prearranged [p, c, o] = weight[c*128+p, o] for a contiguous DMA
    wt = nc.dram_tensor("wt", [P, 2, COUT], f32, kind="ExternalInput")
    degc = nc.dram_tensor("degc", [P, KC], f32, kind="ExternalInput")
    degrt = nc.dram_tensor("degrt", [P, RTILE], f32, kind="ExternalInput")
    out = nc.dram_tensor("out", [RTILE, COUT], f32, kind="ExternalOutput")

    # Y^T r-blocks: 2 channel halves x (512, 512, 256) columns
    RB = [(0, 512), (512, 512), (1024, 256)]

    with (
        _make_tc(nc) as tc,
        tc.tile_pool(name="const", bufs=1) as cp,
        tc.tile_pool(name="atp", bufs=8) as atp,
        tc.tile_pool(name="xchunk", bufs=8) as xcp,
        tc.tile_pool(name="xsp", bufs=8) as xsp,
        tc.tile_pool(name="yts", bufs=4) as ytsp,
        tc.tile_pool(name="outp", bufs=2) as outp,
        tc.tile_pool(name="ytpsum", bufs=1, space="PSUM") as yp,
        tc.tile_pool(name="opsum", bufs=2, space="PSUM") as op,
    ):
        # --- minimal prologue: only what the first matmuls need ---
        degc_sb = cp.tile([P, KC], f32)
        nc.sync.dma_start(out=degc_sb[:], in_=degc[:])
        crec = cp.tile([P, KC], f32)
        nc.vector.reciprocal(crec[:], degc_sb[:])
        dinvc = cp.tile([P, KC], f32)
        nc.scalar.activation(dinvc[:], crec[:],
                             mybir.ActivationFunctionType.Sqrt)

        # --- pipeline: per chunk k, build xs[k] then aggregate
        #     Y^T[h] += xs[k,h].T @ A^T[k] ---
        psYT = []
        for h in range(2):
            for b, (r0, w) in enumerate(RB):
                yt_tile = yp.tile([P, w], f32, tag=f"yt{h}{b}",
                                  name=f"psYT_{h}_{b}")
                psYT.append(yt_tile)
        for k in range(KC):
            xc = xcp.tile([P, CIN], bf16)
            nc.sync.dma_start(out=xc[:], in_=xbf[k * P:(k + 1) * P, :])
            xsk = xsp.tile([P, CIN], bf16, tag="xs")
            nc.vector.tensor_scalar(
                xsk[:], xc[:], dinvc[:, k:k + 1], None,
                mybir.AluOpType.mult)
            atc = atp.tile([P, RTILE], fp8)
            nc.sync.dma_start(out=atc[:], in_=at[k * P:(k + 1) * P, :])
            for h in range(2):
                for b, (r0, w) in enumerate(RB):
                    nc.tensor.matmul(
                        psYT[h * 3 + b][:],
                        lhsT=xsk[:, h * P:(h + 1) * P],
                        rhs=atc[:, r0:r0 + w],
                        start=(k == 0), stop=(k == KC - 1))

        # --- epilogue-only constants, emitted late so their DMAs queue
        #     behind the first streaming chunks ---
        wsb = cp.tile([P, 2, COUT], f32)
        nc.sync.dma_start(out=wsb[:], in_=wt[:])
        wsbr = cp.tile([P, 2, COUT], f32r)
        nc.vector.tensor_copy(wsbr[:], wsb[:])
        # dinv of output rows via ACT only: d^-1/2 = exp(-0.5 * ln d) —
        # nc.vector.reciprocal on [128,1280] costs ~8us of DVE and
        # stalls the xs pipeline; ACT is otherwise idle here
        degrt_sb = cp.tile([P, RTILE], f32)
        nc.sync.dma_start(out=degrt_sb[:], in_=degrt[:])
        lnd = cp.tile([P, RTILE], f32)
        nc.scalar.activation(lnd[:], degrt_sb[:],
                             mybir.ActivationFunctionType.Ln)
        dinvrt = cp.tile([P, RTILE], f32)
        nc.scalar.activation(dinvrt[:], lnd[:],
                             mybir.ActivationFunctionType.Exp, scale=-0.5)

        # --- epilogue per 128-row output tile ---
        for t in range(RT):
            r0 = t * P
            b = r0 // 512
            off = r0 - b * 512
            yts = ytsp.tile([P, 2, P], f32r, tag="yts")
            for h in range(2):
                nc.vector.tensor_tensor(
                    out=yts[:, h, :],
                    in0=psYT[h * 3 + b][:, off:off + P],
                    in1=dinvrt[:, r0:r0 + P],
                    op=mybir.AluOpType.mult)
            psO = op.tile([P, COUT], f32)
            for h in range(2):
                nc.tensor.matmul(psO[:], lhsT=yts[:, h, :],
                                 rhs=wsbr[:, h, :],
                                 start=(h == 0), stop=(h == 1))
            osb = outp.tile([P, COUT], f32)
            nc.vector.tensor_copy(osb[:], psO[:])
            nc.sync.dma_start(out=out[t * P:(t + 1) * P, :], in_=osb[:])

    return nc


def _host_prep(x, weight, edge_index):
    import ml_dtypes

    x = np.asarray(x, dtype=np.float32)
    weight = np.asarray(weight, dtype=np.float32)
    e = np.asarray(edge_index)

    # dedup edges + self-loops, exactly like the reference's dense scatter
    keys = e[0].astype(np.int64) * N + e[1].astype(np.int64)
    diag = np.arange(N, dtype=np.int64) * (N + 1)
    keys = np.unique(np.concatenate([keys, diag]))
    ru = (keys // N).astype(np.int32)
    cu = (keys % N).astype(np.int32)
    deg = np.bincount(ru, minlength=N).astype(np.float32)

    xbf = np.zeros((NPAD, CIN), dtype=ml_dtypes.bfloat16)
    xbf[:N] = x.astype(ml_dtypes.bfloat16)

    degc_flat = np.ones(KC * P, np.float32)
    degc_flat[:N] = deg
    degc = degc_flat.reshape(KC, P).T.copy()   # degc[p, k] = deg[k*128 + p]

    one = ml_dtypes.float8_e4m3(1.0)
    in_maps = []
    core = ru // RPC
    for m in range(NCORES):
        sel = core == m
        r_loc = ru[sel] - m * RPC
        c_src = cu[sel]
        at = np.zeros((NPAD, RTILE), dtype=ml_dtypes.float8_e4m3)
        at[c_src, r_loc] = one

        degr_row = np.ones(RTILE, np.float32)
        degr_row[:RPC] = deg[m * RPC:(m + 1) * RPC]
        degrt = np.broadcast_to(degr_row, (P, RTILE))

        wt_pre = weight.reshape(2, P, COUT).transpose(1, 0, 2)
        in_maps.append({
            "at": at,
            "xbf": xbf,
            "wt": np.ascontiguousarray(wt_pre),
            "degc": degc,
            "degrt": np.ascontiguousarray(degrt),
        })
    return in_maps


def _install_ntff_shim():
    """Provide antenv.axon_hooks (absent in this image) so
    run_bass_kernel_spmd(trace=True) can capture NTFF profiles via the
    axon .so — mirrors trn_agent_boot._ntff_profile_via_ctypes."""
    import types
    if "antenv.axon_hooks" in sys.modules:
        return
    import contextlib
    import ctypes

    so_path = "/opt/axon/libaxon_pjrt.so"
    hook = None
    if os.path.exists(so_path):
        lib = ctypes.CDLL(so_path)
        if hasattr(lib, "axon_start_nrt_profile"):
            lib.axon_start_nrt_profile.argtypes = [
                ctypes.POINTER(ctypes.c_int64), ctypes.c_size_t]
            lib.axon_start_nrt_profile.restype = ctypes.c_int64
            lib.axon_stop_nrt_profile.argtypes = [ctypes.c_char_p]
            lib.axon_stop_nrt_profile.restype = ctypes.c_int64

            @contextlib.contextmanager
            def _hook(output_dir, device_ids):
                import jax
                jax.devices()
                if device_ids:
                    ids = (ctypes.c_int64 * len(device_ids))(*device_ids)
                    rc = lib.axon_start_nrt_profile(ids, len(device_ids))
                else:
                    rc = lib.axon_start_nrt_profile(None, 0)
                if rc != 0:
                    raise RuntimeError(f"axon_start_nrt_profile rc={rc}")
                try:
                    yield
                finally:
                    n = lib.axon_stop_nrt_profile(str(output_dir).encode())
                    print(f"profile: {n} file(s) written to {output_dir}",
                          file=sys.stderr)

            hook = _hook

    mod = types.ModuleType("antenv.axon_hooks")
    mod._hook = hook
    mod.get_axon_ntff_profile_hook = lambda: mod._hook
    mod.set_axon_ntff_profile_hook = lambda h: setattr(mod, "_hook", h)
    sys.modules["antenv.axon_hooks"] = mod


def kernel(x, weight, edge_index, _trace=False):
    from concourse.bass_utils import run_bass_kernel_spmd

    if _trace:
        _install_ntff_shim()

    in_maps = _host_prep(x, weight, edge_index)
    if "nc" not in _BUILD_CACHE:
        _BUILD_CACHE["nc"] = _build()
    nc = _BUILD_CACHE["nc"]

    res = run_bass_kernel_spmd(nc, in_maps, core_ids=list(range(NCORES)),
                               trace=bool(_trace))
    out = np.concatenate(
        [res.results[m]["out"][:RPC] for m in range(NCORES)], axis=0)
    out = np.ascontiguousarray(out, dtype=np.float32)
    if _trace:
        return out, res
    return out


# revision 27
# speedup vs baseline: 1.2402x; 1.2402x over previous
"""GCNConv (dense normalized adjacency) on 8 Trainium2 NeuronCores.

out = D^-1/2 (A + I, deduped) D^-1/2 @ x @ W

Strategy (1D row partition of N across 8 cores, per the sharding hint:
"shard rows of the adjacency and x across devices, replicate the
256x256 weight"):
  - Host: dedup edges + self-loops and encode the binary adjacency
    slice for each core as a dense bf16 0/1 matrix (A^T column-slice,
    [N, 1280]) — an index-data re-encoding (0/1 bitmap) of edge_index,
    no float math. Degrees are integer edge counts (bincount).
    The on-device scatter path (indirect DMA / dma_gather) is precluded
    by this image's neuronxcc: InstDMAGatherAnt fails codegen ("ISA
    wrong length") and per-row indirect DMA issues at ~1.6us/instr.
  - Device (per core, rows r in [1250*m, 1250*m+1250)):
      * dinv = sqrt(1/deg) on DVE+ACT
      * xs = dinv * x, cast to bf16, resident in SBUF
      * stream A^T slice chunks [128, 1280] bf16 from HBM (sequential,
        full DMA bandwidth), aggregate Y[rowtile] += A^T_chunk.T @ xs
        on the PE (79 chunks x 10 rowtiles, fp32 PSUM accumulation)
      * outer scale by dinv[row], PE transpose, project with the
        replicated 256x256 W in fp32, DMA out each 128-row tile.
"""

import os
import sys

sys.path.insert(0, "/opt/trn_rl_repo")
os.environ.setdefault("MYCRO_LOCAL_CACHE", "1")

import numpy as np

N = 10000
NPAD = 10112        # 79 * 128
KC = 79             # node chunks of 128
CIN = 256
COUT = 256
NCORES = 8
RPC = 1250          # valid rows per core
RTILE = 1280        # padded rows per core (10 tiles of 128)
RT = 10
P = 128

_BUILD_CACHE = {}


def _make_tc(nc):
    """TileContext whose instructions carry at most one sync-wait each.

    This image's walrus build encodes a single sync-wait command per
    instruction ("Too many sync wait commands" otherwise); Tile's
    semaphore pass freely attaches several. Split the extras onto
    freshly inserted same-engine nops placed just before the carrying
    instruction, and do the same for the kernel-tail drain.
    """
    import concourse.tile as tile
    from concourse import mybir
    from concourse.vector_clock import ScopedClock

    MAXW = 1

    def make_carrier(engine, chunk):
        nop = mybir.InstNoOp(name=nc.get_next_instruction_name(),
                             ins=[], outs=[])
        nop.engine = engine
        nop.sync_info = mybir.SyncInfo(on_wait=list(chunk), on_update=[])
        return nop

    class TC(tile.TileContext):
        def _add_instruction(self, inst):
            si = getattr(inst, "sync_info", None)
            eng = getattr(inst, "engine", mybir.EngineType.Unassigned)
            if (si is not None and si.on_wait and len(si.on_wait) > MAXW
                    and eng != mybir.EngineType.Unassigned):
                waits = list(si.on_wait)
                movable = [w for w in waits
                           if getattr(w, "wait_reg", None) is None]
                pinned = [w for w in waits
                          if getattr(w, "wait_reg", None) is not None]
                ordered = pinned + movable
                keep, extra = ordered[:MAXW], ordered[MAXW:]
                assert all(getattr(w, "wait_reg", None) is None
                           for w in extra)
                si.on_wait = keep
                for j in range(0, len(extra), MAXW):
                    super()._add_instruction(
                        make_carrier(eng, extra[j:j + MAXW]))
            super()._add_instruction(inst)

        def _drain_and_barrier(self, tick_clock, wait_clock):
            drain_inst = nc.sync.drain()
            wait_clock.add_sem_waits(
                drain_inst.ins, ScopedClock({None: tick_clock.global_clock}))
            raw = drain_inst.ins
            si = raw.sync_info
            waits = list(si.on_wait) if si and si.on_wait else []
            if len(waits) > MAXW:
                bb = nc.cur_bb.bb
                insts = list(bb.instructions)
                assert insts and insts[-1].name == raw.name
                keep, extra = waits[:MAXW], waits[MAXW:]
                si.on_wait = keep
                carriers = []
                for j in range(0, len(extra), MAXW):
                    nop = make_carrier(raw.engine, extra[j:j + MAXW])
                    nc.register_instruction(nop, overwrite=True)
                    carriers.append(nop)
                bb.instructions = insts[:-1] + carriers + [raw]
            nc.all_engine_barrier()
            assert self.sems is not None
            popped = nc._tile_sem_poison_stack.pop()
            assert popped is self._sem_poison
            nc.clear_and_free_semaphores(
                list(self.sems.allocated().values()))
            nc.all_engine_barrier()

    return TC(nc)


def _build():
    import concourse.bass as bass
    from concourse import mybir

    f32 = mybir.dt.float32
    f32r = mybir.dt.float32r
    bf16 = mybir.dt.bfloat16
    fp8 = mybir.dt.float8e4

    nc = bass.Bass("TRN2")

    at = nc.dram_tensor("at", [NPAD, RTILE], fp8, kind="ExternalInput")
    xbf = nc.dram_tensor("xbf", [NPAD, CIN], bf16, kind="ExternalInput")
    # host-prearranged [p, c, o] = weight[c*128+p, o] for a contiguous DMA
    wt = nc.dram_tensor("wt", [P, 2, COUT], f32, kind="ExternalInput")
    degc = nc.dram_tensor("degc", [P, KC], f32, kind="ExternalInput")
    degrt = nc.dram_tensor("degrt", [P, RTILE], f32, kind="ExternalInput")
    out = nc.dram_tensor("out", [RTILE, COUT], f32, kind="ExternalOutput")

    # Y^T r-blocks: 2 channel halves x (512, 512, 256) columns
    RB = [(0, 512), (512, 512), (1024, 256)]

    with (
        _make_tc(nc) as tc,
        tc.tile_pool(name="const", bufs=1) as cp,
        tc.tile_pool(name="atp", bufs=12) as atp,
        tc.tile_pool(name="xchunk", bufs=8) as xcp,
        tc.tile_pool(name="xsp", bufs=8) as xsp,
        tc.tile_pool(name="yts", bufs=4) as ytsp,
        tc.tile_pool(name="outp", bufs=2) as outp,
        tc.tile_pool(name="ytpsum", bufs=1, space="PSUM") as yp,
        tc.tile_pool(name="opsum", bufs=2, space="PSUM") as op,
    ):
        # --- minimal prologue: only what the first matmuls need ---
        degc_sb = cp.tile([P, KC], f32)
        nc.scalar.dma_start(out=degc_sb[:], in_=degc[:])
        crec = cp.tile([P, KC], f32)
        nc.vector.reciprocal(crec[:], degc_sb[:])
        dinvc = cp.tile([P, KC], f32)
        nc.scalar.activation(dinvc[:], crec[:],
                             mybir.ActivationFunctionType.Sqrt)

        # --- pipeline: per chunk k, build xs[k] then aggregate
        #     Y^T[h] += xs[k,h].T @ A^T[k] ---
        psYT = []
        for h in range(2):
            for b, (r0, w) in enumerate(RB):
                yt_tile = yp.tile([P, w], f32, tag=f"yt{h}{b}",
                                  name=f"psYT_{h}_{b}")
                psYT.append(yt_tile)
        for k in range(KC):
            xc = xcp.tile([P, CIN], bf16)
            nc.scalar.dma_start(out=xc[:], in_=xbf[k * P:(k + 1) * P, :])
            xsk = xsp.tile([P, CIN], bf16, tag="xs")
            nc.vector.tensor_scalar(
                xsk[:], xc[:], dinvc[:, k:k + 1], None,
                mybir.AluOpType.mult)
            atc = atp.tile([P, RTILE], fp8)
            nc.sync.dma_start(out=atc[:], in_=at[k * P:(k + 1) * P, :])
            for h in range(2):
                for b, (r0, w) in enumerate(RB):
                    nc.tensor.matmul(
                        psYT[h * 3 + b][:],
                        lhsT=xsk[:, h * P:(h + 1) * P],
                        rhs=atc[:, r0:r0 + w],
                        start=(k == 0), stop=(k == KC - 1))

        # --- epilogue-only constants, emitted late so their DMAs queue
        #     behind the first streaming chunks ---
        wsb = cp.tile([P, 2, COUT], f32)
        nc.scalar.dma_start(out=wsb[:], in_=wt[:])
        wsbr = cp.tile([P, 2, COUT], f32r)
        nc.vector.tensor_copy(wsbr[:], wsb[:])
        # dinv of output rows via ACT only: d^-1/2 = exp(-0.5 * ln d) —
        # nc.vector.reciprocal on [128,1280] costs ~8us of DVE and
        # stalls the xs pipeline; ACT is otherwise idle here
        degrt_sb = cp.tile([P, RTILE], f32)
        nc.scalar.dma_start(out=degrt_sb[:], in_=degrt[:])
        lnd = cp.tile([P, RTILE], f32)
        nc.scalar.activation(lnd[:], degrt_sb[:],
                             mybir.ActivationFunctionType.Ln)
        dinvrt = cp.tile([P, RTILE], f32)
        nc.scalar.activation(dinvrt[:], lnd[:],
                             mybir.ActivationFunctionType.Exp, scale=-0.5)

        # --- epilogue per 128-row output tile ---
        for t in range(RT):
            r0 = t * P
            b = r0 // 512
            off = r0 - b * 512
            yts = ytsp.tile([P, 2, P], f32r, tag="yts")
            for h in range(2):
                nc.vector.tensor_tensor(
                    out=yts[:, h, :],
                    in0=psYT[h * 3 + b][:, off:off + P],
                    in1=dinvrt[:, r0:r0 + P],
                    op=mybir.AluOpType.mult)
            psO = op.tile([P, COUT], f32)
            for h in range(2):
                nc.tensor.matmul(psO[:], lhsT=yts[:, h, :],
                                 rhs=wsbr[:, h, :],
                                 start=(h == 0), stop=(h == 1))
            osb = outp.tile([P, COUT], f32)
            nc.vector.tensor_copy(osb[:], psO[:])
            nc.scalar.dma_start(out=out[t * P:(t + 1) * P, :],
                                in_=osb[:])

    return nc


def _host_prep(x, weight, edge_index):
    import ml_dtypes

    x = np.asarray(x, dtype=np.float32)
    weight = np.asarray(weight, dtype=np.float32)
    e = np.asarray(edge_index)

    # dedup edges + self-loops, exactly like the reference's dense scatter
    keys = e[0].astype(np.int64) * N + e[1].astype(np.int64)
    diag = np.arange(N, dtype=np.int64) * (N + 1)
    keys = np.unique(np.concatenate([keys, diag]))
    ru = (keys // N).astype(np.int32)
    cu = (keys % N).astype(np.int32)
    deg = np.bincount(ru, minlength=N).astype(np.float32)

    xbf = np.zeros((NPAD, CIN), dtype=ml_dtypes.bfloat16)
    xbf[:N] = x.astype(ml_dtypes.bfloat16)

    degc_flat = np.ones(KC * P, np.float32)
    degc_flat[:N] = deg
    degc = degc_flat.reshape(KC, P).T.copy()   # degc[p, k] = deg[k*128 + p]

    one = ml_dtypes.float8_e4m3(1.0)
    in_maps = []
    core = ru // RPC
    for m in range(NCORES):
        sel = core == m
        r_loc = ru[sel] - m * RPC
        c_src = cu[sel]
        at = np.zeros((NPAD, RTILE), dtype=ml_dtypes.float8_e4m3)
        at[c_src, r_loc] = one

        degr_row = np.ones(RTILE, np.float32)
        degr_row[:RPC] = deg[m * RPC:(m + 1) * RPC]
        degrt = np.broadcast_to(degr_row, (P, RTILE))

        wt_pre = weight.reshape(2, P, COUT).transpose(1, 0, 2)
        in_maps.append({
            "at": at,
            "xbf": xbf,
            "wt": np.ascontiguousarray(wt_pre),
            "degc": degc,
            "degrt": np.ascontiguousarray(degrt),
        })
    return in_maps


def _install_ntff_shim():
    """Provide antenv.axon_hooks (absent in this image) so
    run_bass_kernel_spmd(trace=True) can capture NTFF profiles via the
    axon .so — mirrors trn_agent_boot._ntff_profile_via_ctypes."""
    import types
    if "antenv.axon_hooks" in sys.modules:
        return
    import contextlib
    import ctypes

    so_path = "/opt/axon/libaxon_pjrt.so"
    hook = None
    if os.path.exists(so_path):
        lib = ctypes.CDLL(so_path)
        if hasattr(lib, "axon_start_nrt_profile"):
            lib.axon_start_nrt_profile.argtypes = [
                ctypes.POINTER(ctypes.c_int64), ctypes.c_size_t]
            lib.axon_start_nrt_profile.restype = ctypes.c_int64
            lib.axon_stop_nrt_profile.argtypes = [ctypes.c_char_p]
            lib.axon_stop_nrt_profile.restype = ctypes.c_int64

            @contextlib.contextmanager
            def _hook(output_dir, device_ids):
                import jax
                jax.devices()
                if device_ids:
                    ids = (ctypes.c_int64 * len(device_ids))(*device_ids)
                    rc = lib.axon_start_nrt_profile(ids, len(device_ids))
                else:
                    rc = lib.axon_start_nrt_profile(None, 0)
                if rc != 0:
                    raise RuntimeError(f"axon_start_nrt_profile rc={rc}")
                try:
                    yield
                finally:
                    n = lib.axon_stop_nrt_profile(str(output_dir).encode())
                    print(f"profile: {n} file(s) written to {output_dir}",
                          file=sys.stderr)

            hook = _hook

    mod = types.ModuleType("antenv.axon_hooks")
    mod._hook = hook
    mod.get_axon_ntff_profile_hook = lambda: mod._hook
    mod.set_axon_ntff_profile_hook = lambda h: setattr(mod, "_hook", h)
    sys.modules["antenv.axon_hooks"] = mod


def kernel(x, weight, edge_index, _trace=False):
    from concourse.bass_utils import run_bass_kernel_spmd

    if _trace:
        _install_ntff_shim()

    in_maps = _host_prep(x, weight, edge_index)
    if "nc" not in _BUILD_CACHE:
        _BUILD_CACHE["nc"] = _build()
    nc = _BUILD_CACHE["nc"]

    res = run_bass_kernel_spmd(nc, in_maps, core_ids=list(range(NCORES)),
                               trace=bool(_trace))
    out = np.concatenate(
        [res.results[m]["out"][:RPC] for m in range(NCORES)], axis=0)
    out = np.ascontiguousarray(out, dtype=np.float32)
    if _trace:
        return out, res
    return out
